# revision 5
# baseline (speedup 1.0000x reference)
"""GQA attention (RoPE, causal) on 8 TRN2 NeuronCores.

Sharding: core c = (b, g) with b = c // 4 (batch), g = c % 4 (kv-group).
Each core computes 4 query heads + 1 kv head of attention for one batch
element, plus its slice of the output projection; the host sums the 4
partial outputs per batch (row-parallel wo unshard).

Layout tricks:
- wq/wk columns are permuted on the host (per-head de-interleave of RoPE
  pairs). Scores are invariant to a shared per-head permutation of
  head_dim, and this makes RoPE contiguous-block elementwise ops.
- Scores are computed transposed, ST[k, t] = K_rot @ Q_rot^T, so the AV
  matmul consumes exp(ST) directly with V in natural [t, d] layout and a
  ones-column in V yields the softmax denominators for free.
- All big matmuls run in float32r (TF32-like, 11 mantissa bits, full PE
  speed at N>=256); rel err ~1e-4.
"""
import sys

sys.path.insert(0, "/opt/trn_rl_repo")
import numpy as np

import concourse.bass as bass  # noqa: F401
import concourse.tile as tile
from concourse import bacc, mybir
from concourse.bass_utils import run_bass_kernel_spmd

F32 = mybir.dt.float32
F32R = mybir.dt.float32r

B, T, DIM = 2, 2048, 1024
H, KV, HD = 16, 4, 64
NQ = H // KV          # q heads per core
THETA = 10000.0
SCALE = HD ** -0.5
NCORES = 8
QCH = 512             # q-chunk (free dim of scores/AV matmuls)
NQC = T // QCH        # 4 q-chunks
NKT = T // 128        # 16 k-tiles


def build_nc():
    nc = bacc.Bacc(None, target_bir_lowering=False)
    x_d = nc.declare_dram_parameter("x", [T, DIM], F32, isOutput=False)
    wqkv_d = nc.declare_dram_parameter("wqkv", [DIM, 384], F32, isOutput=False)
    wo_d = nc.declare_dram_parameter("wo", [256, DIM], F32, isOutput=False)
    cosq_d = nc.declare_dram_parameter("cosq", [128, T], F32, isOutput=False)
    sinq_d = nc.declare_dram_parameter("sinq", [128, T], F32, isOutput=False)
    triu_d = nc.declare_dram_parameter("triu", [128, 128], F32, isOutput=False)
    id_d = nc.declare_dram_parameter("ident", [128, 128], F32, isOutput=False)
    out_d = nc.declare_dram_parameter("out", [T, DIM], F32, isOutput=True)

    with tile.TileContext(nc) as tc:
        with (
            tc.tile_pool(name="persist", bufs=1) as pp,
            tc.tile_pool(name="vpool", bufs=16) as vp,
            tc.tile_pool(name="ptrans", bufs=2, space="PSUM") as ptr,
            tc.tile_pool(name="pproj", bufs=2, space="PSUM") as ppj,
            tc.tile_pool(name="pst", bufs=2, space="PSUM") as pst,
            tc.tile_pool(name="pav", bufs=2, space="PSUM") as pav,
        ):
            # ---- persistent tiles ----
            cosq = pp.tile([128, T], F32, tag="cosq")
            sinq = pp.tile([128, T], F32, tag="sinq")
            triu = pp.tile([128, 128], F32, tag="triu")
            id_s = pp.tile([128, 128], F32, tag="ident")
            nc.sync.dma_start(cosq[:], cosq_d[:])
            nc.sync.dma_start(sinq[:], sinq_d[:])
            nc.sync.dma_start(triu[:], triu_d[:])
            nc.sync.dma_start(id_s[:], id_d[:])

            wo_r = pp.tile([128, 2, DIM], F32R, tag="wo_r")
            qrot = [
                pp.tile([128, T], F32R, tag="qrot01", name="qrot01"),
                pp.tile([128, T], F32R, tag="qrot23", name="qrot23"),
            ]
            krot2 = pp.tile([128, T], F32R, tag="krot2")
            outTn = [
                pp.tile([128, T], F32R, tag="outTn01", name="outTn01"),
                pp.tile([128, T], F32R, tag="outTn23", name="outTn23"),
            ]
            v_tiles = [
                vp.tile([128, HD + 1], F32R, tag="v", name=f"v{i}")
                for i in range(NKT)
            ]
            ones128 = pp.tile([128, 1], F32, tag="ones128")
            nc.vector.memset(ones128[:], 1.0)

            # ---- phase A: weights round, x transpose, qkv projection, rope ----
            with tc.tile_pool(name="phaseA", bufs=1) as pa, tc.tile_pool(
                name="xio", bufs=5
            ) as pxio, tc.tile_pool(name="xtc", bufs=10) as pxt:
                wqkv_raw = pa.tile([128, 8, 384], F32, tag="sA")
                nc.sync.dma_start(
                    wqkv_raw[:], wqkv_d.rearrange("(k p) c -> p k c", p=128)
                )
                wqkv_r = pa.tile([128, 8, 384], F32R, tag="wqkv_r")
                nc.vector.tensor_copy(wqkv_r[:], wqkv_raw[:])

                wo_raw = pa.tile([128, 2, DIM], F32, tag="sB")
                nc.sync.dma_start(wo_raw[:], wo_d.rearrange("(k p) c -> p k c", p=128))
                nc.vector.tensor_copy(wo_r[:], wo_raw[:])

                t0 = pa.tile([128, T], F32, tag="t0")   # q evens (pre-rope)
                t1 = pa.tile([128, T], F32, tag="t1")   # q odds
                tk = pa.tile([64, T], F32, tag="tk")    # k evens/odds
                vT = pa.tile([64, T], F32, tag="vT")    # v transposed

                for nch in range(NQC):
                    cs = slice(nch * QCH, (nch + 1) * QCH)
                    # load 4 x row-tiles, transpose into xTc[d] [128(dim), 512(t)]
                    xin = [
                        pxio.tile([128, DIM], F32, tag="xin", name=f"xin{i}")
                        for i in range(4)
                    ]
                    for i in range(4):
                        r0 = nch * QCH + i * 128
                        nc.sync.dma_start(xin[i][:], x_d[r0 : r0 + 128, :])
                    xtc = [
                        pxt.tile([128, QCH], F32R, tag="xtc", name=f"xtc{d}")
                        for d in range(8)
                    ]
                    for d in range(8):
                        for i in range(4):
                            ptx = ptr.tile([128, 128], F32, tag="ptx")
                            nc.tensor.transpose(
                                ptx[:], xin[i][:, d * 128 : (d + 1) * 128], id_s[:]
                            )
                            dst = xtc[d][:, i * 128 : (i + 1) * 128]
                            if (d * 4 + i) % 2 == 0:
                                nc.vector.tensor_copy(dst, ptx[:])
                            else:
                                nc.scalar.copy(dst, ptx[:])
                    # projection: 3 M-tiles over this chunk
                    for m in range(3):
                        pq = ppj.tile([128, QCH], F32, tag="pq")
                        for k in range(8):
                            nc.tensor.matmul(
                                pq[:],
                                wqkv_r[:, k, m * 128 : (m + 1) * 128],
                                xtc[k][:],
                                start=(k == 0),
                                stop=(k == 7),
                            )
                        if m == 0:
                            nc.scalar.copy(t0[:, cs], pq[:])
                        elif m == 1:
                            nc.scalar.copy(t1[:, cs], pq[:])
                        else:
                            nc.vector.tensor_copy(tk[0:64, cs], pq[0:64, :])
                            nc.vector.tensor_copy(vT[:, cs], pq[64:128, :])

                # ---- RoPE (q, full width) ----
                sA = pa.tile([128, T], F32, tag="sA")
                sB = pa.tile([128, T], F32, tag="sB")
                nc.vector.tensor_mul(sA[:], t0[:], cosq[:])
                nc.vector.tensor_mul(sB[:], t1[:], sinq[:])
                nc.vector.tensor_sub(sA[:], sA[:], sB[:])      # rotated evens
                nc.vector.tensor_mul(sB[:], t0[:], sinq[:])
                nc.vector.tensor_mul(t0[:], t1[:], cosq[:])
                nc.vector.tensor_add(sB[:], sB[:], t0[:])      # rotated odds
                for h in range(NQ):
                    qt = qrot[h // 2]
                    roff = 64 * (h % 2)
                    src_e = sA[32 * h : 32 * h + 32, :]
                    src_o = sB[32 * h : 32 * h + 32, :]
                    if h % 2 == 0:
                        nc.vector.tensor_copy(qt[roff : roff + 32, :], src_e)
                        nc.scalar.copy(qt[roff + 32 : roff + 64, :], src_o)
                    else:
                        nc.scalar.copy(qt[roff : roff + 32, :], src_e)
                        nc.vector.tensor_copy(qt[roff + 32 : roff + 64, :], src_o)

                # ---- RoPE (k) into krot2 rows 0:64, duplicated to 64:128 ----
                nc.vector.tensor_mul(sA[0:32, :], tk[0:32, :], cosq[0:32, :])
                nc.vector.tensor_mul(sB[0:32, :], tk[32:64, :], sinq[32:64, :])
                nc.vector.tensor_sub(krot2[0:32, :], sA[0:32, :], sB[0:32, :])
                nc.vector.tensor_mul(sA[0:32, :], tk[0:32, :], sinq[0:32, :])
                nc.vector.tensor_mul(sB[0:32, :], tk[32:64, :], cosq[32:64, :])
                nc.vector.tensor_add(krot2[32:64, :], sA[0:32, :], sB[0:32, :])
                nc.vector.tensor_copy(krot2[64:128, :], krot2[0:64, :])

                # ---- V tiles [128, 65] (ones column for denominators) ----
                for i in range(NKT):
                    pv = ptr.tile([128, HD], F32, tag="ptx", name="pv")
                    nc.tensor.transpose(
                        pv[:], vT[:, i * 128 : (i + 1) * 128], id_s[0:64, 0:64]
                    )
                    nc.vector.tensor_copy(v_tiles[i][:, 0:HD], pv[:])
                    nc.scalar.copy(v_tiles[i][:, HD : HD + 1], ones128[:])

            # ---- phase B: attention ----
            with tc.tile_pool(name="phaseB", bufs=3) as pb, tc.tile_pool(
                name="phaseB2", bufs=2
            ) as pb2:
                for h in range(NQ):
                    qt = qrot[h // 2]
                    base = 64 * (h % 2)
                    for qc in range(NQC):
                        nkt = 4 * qc + 4
                        po = pav.tile([HD + 1, QCH], F32, tag="po")
                        for kt in range(nkt):
                            j = kt - 4 * qc
                            col0 = 128 * j if j >= 0 else 0
                            ncols = QCH - col0
                            ps = pst.tile([128, QCH], F32, tag="ps")
                            nc.tensor.matmul(
                                ps[:, col0:QCH],
                                krot2[base : base + 64, kt * 128 : (kt + 1) * 128],
                                qt[base : base + 64, qc * QCH + col0 : (qc + 1) * QCH],
                                start=True,
                                stop=True,
                                tile_position=(base, 0),
                            )
                            et = pb.tile([128, QCH], F32R, tag="et")
                            nc.scalar.activation(
                                et[:, col0:QCH],
                                ps[:, col0:QCH],
                                mybir.ActivationFunctionType.Exp,
                                scale=SCALE,
                            )
                            if j >= 0:
                                nc.vector.tensor_mul(
                                    et[:, col0 : col0 + 128],
                                    et[:, col0 : col0 + 128].bitcast(F32),
                                    triu[:],
                                )
                            nc.tensor.matmul(
                                po[:, col0:QCH],
                                v_tiles[kt][:],
                                et[:, col0:QCH],
                                start=(kt == 0),
                                stop=(kt == nkt - 1),
                            )
                        # normalize: out[d,t] / den[t]
                        o65 = pb2.tile([HD + 1, QCH], F32, tag="o65")
                        nc.scalar.copy(o65[:], po[:])
                        lnv = pb2.tile([1, QCH], F32, tag="lnv")
                        nc.scalar.activation(
                            lnv[:], o65[HD : HD + 1, :],
                            mybir.ActivationFunctionType.Ln,
                        )
                        inv = pb2.tile([1, QCH], F32, tag="inv")
                        nc.scalar.activation(
                            inv[:], lnv[:],
                            mybir.ActivationFunctionType.Exp, scale=-1.0,
                        )
                        bcast = pb2.tile([HD, QCH], F32, tag="bcast")
                        nc.gpsimd.partition_broadcast(bcast[:], inv[:])
                        nc.vector.tensor_mul(
                            outTn[h // 2][base : base + 64, qc * QCH : (qc + 1) * QCH],
                            o65[0:HD, :],
                            bcast[:],
                        )

                # ---- output projection: y = outTn.T @ wo ----
                for tq in range(NKT):
                    for n2 in range(2):
                        py = ppj.tile([128, QCH], F32, tag="pq", name="py")
                        nc.tensor.matmul(
                            py[:],
                            outTn[0][:, tq * 128 : (tq + 1) * 128],
                            wo_r[:, 0, n2 * QCH : (n2 + 1) * QCH],
                            start=True,
                            stop=False,
                        )
                        nc.tensor.matmul(
                            py[:],
                            outTn[1][:, tq * 128 : (tq + 1) * 128],
                            wo_r[:, 1, n2 * QCH : (n2 + 1) * QCH],
                            start=False,
                            stop=True,
                        )
                        ys = pb2.tile([128, QCH], F32, tag="ys")
                        nc.vector.tensor_copy(ys[:], py[:])
                        nc.sync.dma_start(
                            out_d[tq * 128 : (tq + 1) * 128, n2 * QCH : (n2 + 1) * QCH],
                            ys[:],
                        )
    nc.compile()
    return nc


def _host_tables():
    ev = np.arange(0, HD, 2)
    od = ev + 1
    inv = 1.0 / (THETA ** (np.arange(0, HD, 2, dtype=np.float64) / HD))  # [32]
    freqs = np.outer(inv, np.arange(T, dtype=np.float64))  # [32, T]
    cosq = np.tile(np.cos(freqs), (4, 1)).astype(np.float32)  # [128, T]
    sinq = np.tile(np.sin(freqs), (4, 1)).astype(np.float32)
    triu = np.triu(np.ones((128, 128), np.float32))
    ident = np.eye(128, dtype=np.float32)
    return ev, od, cosq, sinq, triu, ident


_NC_CACHE = None


def kernel(**inputs):
    global _NC_CACHE
    x = np.asarray(inputs["x"], dtype=np.float32)
    wq = np.asarray(inputs["wq"], dtype=np.float32)
    wk = np.asarray(inputs["wk"], dtype=np.float32)
    wv = np.asarray(inputs["wv"], dtype=np.float32)
    wo = np.asarray(inputs["wo"], dtype=np.float32)

    ev, od, cosq, sinq, triu, ident = _host_tables()

    if _NC_CACHE is None:
        _NC_CACHE = build_nc()
    nc = _NC_CACHE

    in_maps = []
    for c in range(NCORES):
        b, g = c // 4, c % 4
        qe = np.concatenate([wq[:, 64 * (4 * g + h) + ev] for h in range(NQ)], axis=1)
        qo = np.concatenate([wq[:, 64 * (4 * g + h) + od] for h in range(NQ)], axis=1)
        wqkv_g = np.concatenate(
            [
                qe,
                qo,
                wk[:, 64 * g + ev],
                wk[:, 64 * g + od],
                wv[:, 64 * g : 64 * (g + 1)],
            ],
            axis=1,
        ).astype(np.float32)  # [1024, 384]
        wo_g = np.ascontiguousarray(wo[256 * g : 256 * (g + 1), :], dtype=np.float32)
        in_maps.append(
            {
                "x": np.ascontiguousarray(x[b]),
                "wqkv": np.ascontiguousarray(wqkv_g),
                "wo": wo_g,
                "cosq": cosq,
                "sinq": sinq,
                "triu": triu,
                "ident": ident,
            }
        )

    res = run_bass_kernel_spmd(nc, in_maps, list(range(NCORES)))
    out = np.zeros((B, T, DIM), np.float32)
    for c in range(NCORES):
        out[c // 4] += res.results[c]["out"]
    return out


# revision 9
# speedup vs baseline: 1.1471x; 1.1471x over previous
"""GQA attention (RoPE, causal) on 8 TRN2 NeuronCores.

Sharding: core c = (b, g) with b = c // 4 (batch), g = c % 4 (kv-group).
Each core computes 4 query heads + 1 kv head of attention for one batch
element, plus its slice of the output projection; the host sums the 4
partial outputs per batch (row-parallel wo unshard).

Layout tricks:
- wq/wk columns are permuted on the host (per-head de-interleave of RoPE
  pairs). Scores are invariant to a shared per-head permutation of
  head_dim, and this makes RoPE contiguous-block elementwise ops.
- Scores are computed transposed, ST[k, t] = K_rot @ Q_rot^T, so the AV
  matmul consumes exp(ST) directly with V in natural [t, d] layout and a
  ones-column in V yields the softmax denominators for free.
- Matmuls run in bf16 (f32 accumulate); x and the weights are cast to
  bf16 on the host. Softmax skips the max-subtraction (scores are small,
  exp is safe in f32) and normalization is batched per head.
"""
import sys

sys.path.insert(0, "/opt/trn_rl_repo")
import ml_dtypes
import numpy as np

import concourse.bass as bass  # noqa: F401
import concourse.tile as tile
from concourse import bacc, mybir
from concourse.bass_utils import run_bass_kernel_spmd

F32 = mybir.dt.float32
BF16 = mybir.dt.bfloat16

B, T, DIM = 2, 2048, 1024
H, KV, HD = 16, 4, 64
NQ = H // KV          # q heads per core
THETA = 10000.0
SCALE = HD ** -0.5
NCORES = 8
QCH = 512             # q-chunk (free dim of scores/AV matmuls)
NQC = T // QCH        # 4 q-chunks
NKT = T // 128        # 16 k-tiles


def build_nc():
    nc = bacc.Bacc(None, target_bir_lowering=False)
    x_d = nc.declare_dram_parameter("x", [T, DIM], BF16, isOutput=False)
    wqkv_d = nc.declare_dram_parameter("wqkv", [DIM, 384], BF16, isOutput=False)
    wo_d = nc.declare_dram_parameter("wo", [256, DIM], BF16, isOutput=False)
    cosq_d = nc.declare_dram_parameter("cosq", [128, T], F32, isOutput=False)
    sinq_d = nc.declare_dram_parameter("sinq", [128, T], F32, isOutput=False)
    triu_d = nc.declare_dram_parameter("triu", [128, 128], BF16, isOutput=False)
    id_d = nc.declare_dram_parameter("ident", [128, 128], BF16, isOutput=False)
    out_d = nc.declare_dram_parameter("out", [T, DIM], BF16, isOutput=True)

    with tile.TileContext(nc) as tc:
        with (
            tc.tile_pool(name="persist", bufs=1) as pp,
            tc.tile_pool(name="vpool", bufs=16) as vp,
            tc.tile_pool(name="ptrans", bufs=2, space="PSUM") as ptr,
            tc.tile_pool(name="pproj", bufs=2, space="PSUM") as ppj,
            tc.tile_pool(name="pst", bufs=2, space="PSUM") as pst,
            tc.tile_pool(name="pav", bufs=2, space="PSUM") as pav,
        ):
            # ---- persistent tiles ----
            cosq = pp.tile([128, T], F32, tag="cosq")
            sinq = pp.tile([128, T], F32, tag="sinq")
            triu = pp.tile([128, 128], BF16, tag="triu")
            id_s = pp.tile([128, 128], BF16, tag="ident")
            nc.sync.dma_start(cosq[:], cosq_d[:])
            nc.sync.dma_start(sinq[:], sinq_d[:])
            nc.sync.dma_start(triu[:], triu_d[:])
            nc.sync.dma_start(id_s[:], id_d[:])

            wqkv_s = pp.tile([128, 8, 384], BF16, tag="wqkv_s")
            nc.sync.dma_start(wqkv_s[:], wqkv_d.rearrange("(k p) c -> p k c", p=128))
            wo_s = pp.tile([128, 2, DIM], BF16, tag="wo_s")
            nc.sync.dma_start(wo_s[:], wo_d.rearrange("(k p) c -> p k c", p=128))

            qrot = [
                pp.tile([128, T], BF16, tag="qrot01", name="qrot01"),
                pp.tile([128, T], BF16, tag="qrot23", name="qrot23"),
            ]
            krot2 = pp.tile([128, T], BF16, tag="krot2")
            outU = [
                pp.tile([128, T], F32, tag="outU01", name="outU01"),
                pp.tile([128, T], F32, tag="outU23", name="outU23"),
            ]
            outTn = [
                pp.tile([128, T], BF16, tag="outTn01", name="outTn01"),
                pp.tile([128, T], BF16, tag="outTn23", name="outTn23"),
            ]
            v_tiles = [
                vp.tile([128, HD + 1], BF16, tag="v", name=f"v{i}")
                for i in range(NKT)
            ]
            ones128 = pp.tile([128, 1], BF16, tag="ones128")
            nc.vector.memset(ones128[:], 1.0)

            # ---- phase A: x transpose, qkv projection, rope, v build ----
            with tc.tile_pool(name="phaseA", bufs=1) as pa, tc.tile_pool(
                name="xio", bufs=5
            ) as pxio, tc.tile_pool(name="xtc", bufs=10) as pxt:
                t0 = pa.tile([128, T], F32, tag="t0")   # q evens (pre-rope)
                t1 = pa.tile([128, T], F32, tag="t1")   # q odds
                tk = pa.tile([64, T], F32, tag="tk")    # k evens/odds
                vT = pa.tile([64, T], BF16, tag="vT")   # v transposed

                for nch in range(NQC):
                    cs = slice(nch * QCH, (nch + 1) * QCH)
                    xin = [
                        pxio.tile([128, DIM], BF16, tag="xin", name=f"xin{i}")
                        for i in range(4)
                    ]
                    for i in range(4):
                        r0 = nch * QCH + i * 128
                        nc.sync.dma_start(xin[i][:], x_d[r0 : r0 + 128, :])
                    xtc = [
                        pxt.tile([128, QCH], BF16, tag="xtc", name=f"xtc{d}")
                        for d in range(8)
                    ]
                    for d in range(8):
                        for i in range(4):
                            ptx = ptr.tile([128, 128], BF16, tag="ptx")
                            nc.tensor.transpose(
                                ptx[:], xin[i][:, d * 128 : (d + 1) * 128], id_s[:]
                            )
                            dst = xtc[d][:, i * 128 : (i + 1) * 128]
                            if (d * 4 + i) % 2 == 0:
                                nc.vector.tensor_copy(dst, ptx[:])
                            else:
                                nc.scalar.copy(dst, ptx[:])
                    # projection: 3 M-tiles over this chunk
                    for m in range(3):
                        pq = ppj.tile([128, QCH], F32, tag="pq")
                        for k in range(8):
                            nc.tensor.matmul(
                                pq[:],
                                wqkv_s[:, k, m * 128 : (m + 1) * 128],
                                xtc[k][:],
                                start=(k == 0),
                                stop=(k == 7),
                            )
                        if m == 0:
                            nc.scalar.copy(t0[:, cs], pq[:])
                        elif m == 1:
                            nc.scalar.copy(t1[:, cs], pq[:])
                        else:
                            nc.vector.tensor_copy(tk[0:64, cs], pq[0:64, :])
                            nc.vector.tensor_copy(vT[:, cs], pq[64:128, :])

                # ---- RoPE (q, full width) ----
                sA = pa.tile([128, T], F32, tag="sA")
                sB = pa.tile([128, T], F32, tag="sB")
                nc.vector.tensor_mul(sA[:], t0[:], cosq[:])
                nc.vector.tensor_mul(sB[:], t1[:], sinq[:])
                nc.vector.tensor_sub(sA[:], sA[:], sB[:])      # rotated evens
                nc.vector.tensor_mul(sB[:], t0[:], sinq[:])
                nc.vector.tensor_mul(t0[:], t1[:], cosq[:])
                nc.vector.tensor_add(sB[:], sB[:], t0[:])      # rotated odds
                for h in range(NQ):
                    qt = qrot[h // 2]
                    roff = 64 * (h % 2)
                    src_e = sA[32 * h : 32 * h + 32, :]
                    src_o = sB[32 * h : 32 * h + 32, :]
                    if h % 2 == 0:
                        nc.vector.tensor_copy(qt[roff : roff + 32, :], src_e)
                        nc.scalar.copy(qt[roff + 32 : roff + 64, :], src_o)
                    else:
                        nc.scalar.copy(qt[roff : roff + 32, :], src_e)
                        nc.vector.tensor_copy(qt[roff + 32 : roff + 64, :], src_o)

                # ---- RoPE (k) into krot2 rows 0:64, duplicated to 64:128 ----
                nc.vector.tensor_mul(sA[0:32, :], tk[0:32, :], cosq[0:32, :])
                nc.vector.tensor_mul(sB[0:32, :], tk[32:64, :], sinq[32:64, :])
                nc.vector.tensor_sub(krot2[0:32, :], sA[0:32, :], sB[0:32, :])
                nc.vector.tensor_mul(sA[0:32, :], tk[0:32, :], sinq[0:32, :])
                nc.vector.tensor_mul(sB[0:32, :], tk[32:64, :], cosq[32:64, :])
                nc.vector.tensor_add(krot2[32:64, :], sA[0:32, :], sB[0:32, :])
                nc.vector.tensor_copy(krot2[64:128, :], krot2[0:64, :])

                # ---- V tiles [128, 65] (ones column for denominators) ----
                for i in range(NKT):
                    pv = ptr.tile([128, HD], BF16, tag="ptx", name="pv")
                    nc.tensor.transpose(
                        pv[:],
                        vT[:, i * 128 : (i + 1) * 128],
                        id_s[0:64, 0:64],
                    )
                    nc.vector.tensor_copy(v_tiles[i][:, 0:HD], pv[:])
                    nc.scalar.copy(v_tiles[i][:, HD : HD + 1], ones128[:])

            # ---- phase B: attention ----
            with tc.tile_pool(name="phaseB", bufs=3) as pb, tc.tile_pool(
                name="phaseB2", bufs=2
            ) as pb2, tc.tile_pool(name="phaseB1", bufs=1) as pb1:
                den = [
                    pb1.tile([1, T], F32, tag=f"den{h}", name=f"den{h}")
                    for h in range(NQ)
                ]
                for h in range(NQ):
                    qt = qrot[h // 2]
                    base = 64 * (h % 2)
                    for qc in range(NQC):
                        nkt = 4 * qc + 4
                        po = pav.tile([HD + 1, QCH], F32, tag="po")
                        for kt in range(nkt):
                            j = kt - 4 * qc
                            col0 = 128 * j if j >= 0 else 0
                            ps = pst.tile([128, QCH], F32, tag="ps")
                            nc.tensor.matmul(
                                ps[:, col0:QCH],
                                krot2[base : base + 64, kt * 128 : (kt + 1) * 128],
                                qt[base : base + 64, qc * QCH + col0 : (qc + 1) * QCH],
                                start=True,
                                stop=True,
                                tile_position=(base, 0),
                            )
                            et = pb.tile([128, QCH], BF16, tag="et")
                            nc.scalar.activation(
                                et[:, col0:QCH],
                                ps[:, col0:QCH],
                                mybir.ActivationFunctionType.Exp,
                                scale=SCALE,
                            )
                            if j >= 0:
                                nc.vector.tensor_mul(
                                    et[:, col0 : col0 + 128],
                                    et[:, col0 : col0 + 128],
                                    triu[:],
                                )
                            nc.tensor.matmul(
                                po[:, col0:QCH],
                                v_tiles[kt][:],
                                et[:, col0:QCH],
                                start=(kt == 0),
                                stop=(kt == nkt - 1),
                            )
                        # stash unnormalized output + denominators
                        o65 = pb2.tile([HD + 1, QCH], F32, tag="o65")
                        nc.scalar.copy(o65[:], po[:])
                        nc.vector.tensor_copy(
                            outU[h // 2][base : base + 64, qc * QCH : (qc + 1) * QCH],
                            o65[0:HD, :],
                        )
                        nc.vector.tensor_copy(
                            den[h][:, qc * QCH : (qc + 1) * QCH],
                            o65[HD : HD + 1, :],
                        )

                # ---- batched normalization (2 ACT table loads total) ----
                inv = [
                    pb1.tile([1, T], F32, tag=f"inv{h}", name=f"inv{h}")
                    for h in range(NQ)
                ]
                for h in range(NQ):
                    nc.scalar.activation(
                        den[h][:], den[h][:], mybir.ActivationFunctionType.Ln
                    )
                for h in range(NQ):
                    nc.scalar.activation(
                        inv[h][:], den[h][:],
                        mybir.ActivationFunctionType.Exp, scale=-1.0,
                    )
                for h in range(NQ):
                    base = 64 * (h % 2)
                    for qc in range(NQC):
                        bc = pb2.tile([128, QCH], F32, tag="bc", name="bc")
                        nc.gpsimd.partition_broadcast(
                            bc[:], inv[h][:, qc * QCH : (qc + 1) * QCH]
                        )
                        nc.vector.tensor_mul(
                            outTn[h // 2][base : base + 64, qc * QCH : (qc + 1) * QCH],
                            outU[h // 2][base : base + 64, qc * QCH : (qc + 1) * QCH],
                            bc[base : base + 64, :],
                        )

                # ---- output projection: y = outTn.T @ wo ----
                for tq in range(NKT):
                    for n2 in range(2):
                        py = ppj.tile([128, QCH], F32, tag="pq", name="py")
                        nc.tensor.matmul(
                            py[:],
                            outTn[0][:, tq * 128 : (tq + 1) * 128],
                            wo_s[:, 0, n2 * QCH : (n2 + 1) * QCH],
                            start=True,
                            stop=False,
                        )
                        nc.tensor.matmul(
                            py[:],
                            outTn[1][:, tq * 128 : (tq + 1) * 128],
                            wo_s[:, 1, n2 * QCH : (n2 + 1) * QCH],
                            start=False,
                            stop=True,
                        )
                        ys = pb2.tile([128, QCH], BF16, tag="ys")
                        nc.vector.tensor_copy(ys[:], py[:])
                        nc.sync.dma_start(
                            out_d[tq * 128 : (tq + 1) * 128, n2 * QCH : (n2 + 1) * QCH],
                            ys[:],
                        )
    nc.compile()
    return nc


def _host_tables():
    ev = np.arange(0, HD, 2)
    od = ev + 1
    inv = 1.0 / (THETA ** (np.arange(0, HD, 2, dtype=np.float64) / HD))  # [32]
    freqs = np.outer(inv, np.arange(T, dtype=np.float64))  # [32, T]
    cosq = np.tile(np.cos(freqs), (4, 1)).astype(np.float32)  # [128, T]
    sinq = np.tile(np.sin(freqs), (4, 1)).astype(np.float32)
    triu = np.triu(np.ones((128, 128), np.float32)).astype(ml_dtypes.bfloat16)
    ident = np.eye(128, dtype=np.float32).astype(ml_dtypes.bfloat16)
    return ev, od, cosq, sinq, triu, ident


def make_in_maps(inputs):
    x = np.asarray(inputs["x"], dtype=np.float32)
    wq = np.asarray(inputs["wq"], dtype=np.float32)
    wk = np.asarray(inputs["wk"], dtype=np.float32)
    wv = np.asarray(inputs["wv"], dtype=np.float32)
    wo = np.asarray(inputs["wo"], dtype=np.float32)
    ev, od, cosq, sinq, triu, ident = _host_tables()
    in_maps = []
    for c in range(NCORES):
        b, g = c // 4, c % 4
        qe = np.concatenate([wq[:, 64 * (4 * g + h) + ev] for h in range(NQ)], axis=1)
        qo = np.concatenate([wq[:, 64 * (4 * g + h) + od] for h in range(NQ)], axis=1)
        wqkv_g = np.concatenate(
            [
                qe,
                qo,
                wk[:, 64 * g + ev],
                wk[:, 64 * g + od],
                wv[:, 64 * g : 64 * (g + 1)],
            ],
            axis=1,
        ).astype(ml_dtypes.bfloat16)  # [1024, 384]
        wo_g = wo[256 * g : 256 * (g + 1), :].astype(ml_dtypes.bfloat16)
        in_maps.append(
            {
                "x": np.ascontiguousarray(x[b]).astype(ml_dtypes.bfloat16),
                "wqkv": np.ascontiguousarray(wqkv_g),
                "wo": np.ascontiguousarray(wo_g),
                "cosq": cosq,
                "sinq": sinq,
                "triu": triu,
                "ident": ident,
            }
        )
    return in_maps


_NC_CACHE = None


def kernel(**inputs):
    global _NC_CACHE
    if _NC_CACHE is None:
        _NC_CACHE = build_nc()
    in_maps = make_in_maps(inputs)
    res = run_bass_kernel_spmd(_NC_CACHE, in_maps, list(range(NCORES)))
    out = np.zeros((B, T, DIM), np.float32)
    for c in range(NCORES):
        out[c // 4] += np.asarray(res.results[c]["out"], dtype=np.float32)
    return out
